# revision 8
# baseline (speedup 1.0000x reference)
"""GAT layer (dense adjacency) on 8 Trainium2 NeuronCores.

Problem: H = elu(softmax_j(mask(A, leaky_relu(Wh1_i + Wh2_j))) @ Wh),
A: [8, 2048, 2048] 0/1 f32, X: [8, 2048, 64], Ws: [64, 64], a: [128, 1].

Sharding: data-parallel over batch B=8 -> one batch element per core.

Single pass over 16 j-slabs (A column tiles), paced by the A DMA
(~2.9 us per 1 MiB slab at 360 GB/s).  All per-slab compute is balanced
under the DMA pace by spreading it over four engines:
  - PE builds the full masked logits in PSUM (4 single-bank chunks of
    512 i-cols, ring of 5 banks):
        z[j, i] = Wh1[i] + (Wh2[j] - C) + C*A[i, j]
    via one K=4 matmul (rows: wh1_hi, wh1_lo broadcast by ones; ones
    rows scaled by (Wh2-C) hi/lo in the stationary operand) plus four
    K=128 matmuls with C*I against the bf16 high halves of the f32 A
    slab (the matmul transposes A; contraction depth is free).
  - leaky_relu(z) is computed chunk-parallel on ACT and DVE into one
    contiguous fp16 SBUF tile e_all[128, 2048]:
      cols [0:640]: ACT Prelu(z) (chunk 0 + first 128 of chunk 1)
      cols [640:2048]: DVE z2 = 0.2*z (fp16 out, the one PSUM-reading
      pass), then max(z, 0.2*z) ~= (z2*5) max z2 as one fused
      scalar_tensor_tensor on fp16 SBUF operands (2x DVE mode).  A
      direct (z*0.2) max z would need two PSUM reads in one op (PSUM
      has a single DVE read port) and GPSIMD cannot run tensor ops.
  - One ACT Exp over all 2048 cols: pa = exp(e_all - S) (fp16).  The
    masked entries (z ~= e - C) underflow to 0, implementing the mask.
  - H accumulates on PE in natural [i, d] layout: per i-tile m,
    acc[:, col(m):col(m)+65] += pa[:, 128m:128(m+1)]^T @ [Wh | 1]
    (ones column gives the softmax row sums).  3 PSUM banks, 7/7/2.
  - The last slab splits its Exp into three pieces aligned with the
    accumulator banks so each bank's epilogue starts early.
  - Epilogue per bank: strided reciprocals -> rc; y = acc * rc (DVE);
    elu via elu(y) = max(min(e^y - 1, 0), y): Exp (ACT), min (DVE TSP),
    max (DVE TT); one fp16 output DMA per bank into a packed [128, 1024]
    DRAM tensor (2 KiB rows, full DMA bandwidth), unpacked on the host.
"""
import sys

for _p in ("/opt/trn_rl_repo",):
    if _p not in sys.path:
        sys.path.append(_p)

import numpy as np
import ml_dtypes

import concourse.bass as bass
import concourse.bacc as bacc
import concourse.tile as tile
from concourse import mybir
from concourse import bass_utils

F32 = mybir.dt.float32
BF16 = mybir.dt.bfloat16
FP16 = mybir.dt.float16
AF = mybir.ActivationFunctionType
ALU = mybir.AluOpType

B, N, F, D = 8, 2048, 64, 64
NT = N // 128          # 16 j-slabs / i-tiles
C_MASK = 512.0
ALPHA = 0.2

_CACHED = {}


def _acc_col(m):
    """Column offset of i-tile m inside the [128, 1536] PSUM accumulator
    (7 tiles in bank 0, 7 in bank 1, 2 in bank 2)."""
    return 512 * (m // 7) + 65 * (m % 7)


def _build_program():
    nc = bacc.Bacc("TRN2", target_bir_lowering=False, debug=False)

    A_d = nc.dram_tensor("A", [N, N], F32, kind="ExternalInput")
    blobS_d = nc.dram_tensor("blobS", [128, 128], F32, kind="ExternalInput")
    blob2_d = nc.dram_tensor("blob2", [4, 4096], BF16, kind="ExternalInput")
    blobW_d = nc.dram_tensor("blobW", [128, NT * (D + 1)], FP16,
                             kind="ExternalInput")
    H_d = nc.dram_tensor("H", [128, NT * D], FP16, kind="ExternalOutput")

    with tile.TileContext(nc) as tc:
        with tc.tile_pool(name="const", bufs=1) as cp, \
             tc.tile_pool(name="aslab", bufs=8) as ap_pool, \
             tc.tile_pool(name="work", bufs=3) as wp, \
             tc.tile_pool(name="outp", bufs=1) as op_pool, \
             tc.tile_pool(name="psP", bufs=5, space="PSUM") as psP, \
             tc.tile_pool(name="psA", bufs=1, space="PSUM") as psA:

            # ---- constants (3 DMAs; small ones first) ----
            cbS = cp.tile([128, 128], F32, name="cbS")
            nc.sync.dma_start(cbS[:], blobS_d.ap())
            cb2 = cp.tile([4, 4096], BF16, name="cb2")
            nc.sync.dma_start(cb2[:], blob2_d.ap())
            cbW = cp.tile([128, NT * (D + 1)], FP16, name="cbW")
            negS = cbS[:, 0:1]
            ci = cbS[:].bitcast(BF16)[:, 128:256]
            fillB = cb2[:, 0:2048]        # rows: wh1_hi, wh1_lo, 1, 1
            fillL = cb2[:, 2048:4096]     # rows: 1, 1, wz_hi, wz_lo
            alpha02 = cp.tile([128, 1], F32, name="alpha02")
            nc.vector.memset(alpha02[:], ALPHA)
            # preload the ACT table set during input DMA
            warm = cp.tile([1, 1], F32, name="warm")
            nc.vector.memset(warm[:], 0.0)
            warm2 = cp.tile([1, 1], F32, name="warm2")
            nc.scalar.activation(warm2[:], warm[:], AF.Exp, bias=0.0, scale=1.0)

            # H^pre accumulator (+ row sums), [i, d] layout, 3 banks.
            # Zeroed once; the accumulating matmuls all use start=False
            # (start=True resets the whole PSUM bank, wiping the slab-0
            # contribution of co-resident regions).
            accs = [psA.tile([128, 512], F32, name=f"acc{b3}")
                    for b3 in range(3)]
            for b3 in range(3):
                nc.vector.memset(accs[b3][:], 0.0)

            aslabs = {}
            state = {}

            def dma_slab(t, split=False):
                sl = ap_pool.tile([128, N], F32, name=f"aslab{t}", tag="aslab")
                # sl[p, 128*r + q] = A[128*r + p, 128*t + q]
                if split:  # 4 row-block chunks so fills can start early
                    for rr in range(4):
                        nc.sync.dma_start(
                            sl[:, 512 * rr:512 * (rr + 1)]
                            .rearrange("p (r q) -> p r q", q=128),
                            A_d.ap()[512 * rr:512 * (rr + 1),
                                     128 * t:128 * (t + 1)]
                            .rearrange("(r p) q -> p r q", p=128),
                        )
                else:
                    nc.sync.dma_start(
                        sl[:].rearrange("p (r q) -> p r q", q=128),
                        A_d.ap()[:, 128 * t:128 * (t + 1)]
                        .rearrange("(r p) q -> p r q", p=128),
                    )
                aslabs[t] = sl

            def fills(t):
                # 4 single-bank chunks of 512 i-cols each; chunk c gets
                # z = Wh1[i] + (Wh2[j]-C) via one K=4 matmul, then + C*A
                # via 4 K=128 matmuls on the bf16 high halves of A.
                chunks = []
                lhs_t = fillL[:, 128 * t:128 * (t + 1)]
                for c in range(4):
                    pp = psP.tile([128, 512], F32, name=f"pp{c}_{t}", tag="pp")
                    nc.tensor.matmul(
                        pp[:], lhs_t, fillB[:, 512 * c:512 * (c + 1)],
                        start=True, stop=False,
                    )
                    for k4 in range(4):
                        r = 4 * c + k4
                        nc.tensor.matmul(
                            pp[:, 128 * k4:128 * (k4 + 1)],
                            aslabs[t][:].bitcast(BF16)[:, 256 * r + 1:256 * (r + 1):2],
                            ci[:],
                            start=False, stop=True, skip_group_check=True,
                        )
                    chunks.append(pp)
                return chunks

            def leaky(t, pp):
                # e_all[128, 2048] fp16 = leaky_relu(z), assembled
                # chunk-parallel across ACT and DVE.
                e_all = wp.tile([128, 2048], FP16, name=f"ea{t}", tag="ea")
                nc.scalar.activation(
                    e_all[:, 0:512], pp[0][:], AF.Prelu,
                    bias=0.0, scale=1.0, alpha=alpha02[:])
                nc.scalar.activation(
                    e_all[:, 512:640], pp[1][:, 0:128], AF.Prelu,
                    bias=0.0, scale=1.0, alpha=alpha02[:])
                for (c, c0, c1) in ((1, 128, 512), (2, 0, 512), (3, 0, 512)):
                    w = c1 - c0
                    z2 = wp.tile([128, w], FP16, name=f"z2_{t}_{c}",
                                 tag=f"z2{c}")
                    nc.vector.tensor_scalar(
                        z2[:], pp[c][:, c0:c1], ALPHA, None, ALU.mult)
                    nc.vector.scalar_tensor_tensor(
                        e_all[:, 512 * c + c0:512 * c + c1],
                        z2[:], 1.0 / ALPHA, z2[:], ALU.mult, ALU.max)
                state[t] = e_all

            def exp(t):
                # pa = exp(e_all - S); last slab split on accumulator-bank
                # boundaries (i-tiles 0-6 / 7-13 / 14-15).
                e_all = state.pop(t)
                pa = wp.tile([128, 2048], FP16, name=f"pa{t}", tag="pa",
                             bufs=4)
                if t == NT - 1:
                    for (c0, c1) in ((0, 896), (896, 1792), (1792, 2048)):
                        nc.scalar.activation(
                            pa[:, c0:c1], e_all[:, c0:c1], AF.Exp,
                            bias=negS, scale=1.0)
                else:
                    nc.scalar.activation(
                        pa[:], e_all[:], AF.Exp, bias=negS, scale=1.0)
                return pa

            def accum(t, pa):
                for m in range(NT):
                    c0 = 65 * (m % 7)
                    nc.tensor.matmul(
                        accs[m // 7][:, c0:c0 + D + 1],
                        pa[:, 128 * m:128 * (m + 1)],
                        cbW[:, (D + 1) * t:(D + 1) * (t + 1)],
                        start=False, stop=(t == NT - 1),
                        skip_group_check=True,
                    )

            # ---- software-pipelined main loop ----
            # Lag structure: at step k the ACT queue runs Exp(k-1) then
            # Prelu(k); PE runs fills(k) then accum(k-3); DVE/Pool run
            # leaky(k).  DMA leads by 4 slabs.
            pas = {}
            dma_slab(0, split=True)
            for t in range(1, 4):
                dma_slab(t)
            nc.sync.dma_start(cbW[:], blobW_d.ap())
            for k in range(NT + 4):
                if k + 4 < NT:
                    dma_slab(k + 4)
                if 1 <= k <= NT:
                    pas[k - 1] = exp(k - 1)
                if k < NT:
                    pp = fills(k)
                    leaky(k, pp)
                if 3 <= k < NT + 3:
                    accum(k - 3, pas.pop(k - 3))

            # ---- epilogue per accumulator bank:
            # H = max(min(e^y - 1, 0), y), y = H_pre * (1/s) ----
            banks = [(0, 7), (7, 7), (14, 2)]  # (first tile, count)
            for bi, (m0, g) in enumerate(banks):
                Ab = accs[bi]
                rc_b = op_pool.tile([128, g], F32, name=f"rc{bi}")
                y_b = op_pool.tile([128, g * 64], FP16, name=f"y{bi}")
                W_b = op_pool.tile([128, g * 64], FP16, name=f"W{bi}")
                t1_b = op_pool.tile([128, g * 64], FP16, name=f"t1{bi}")
                h_b = op_pool.tile([128, g * 64], FP16, name=f"h{bi}")
                nc.vector.reciprocal(
                    rc_b[:], Ab[:, 64:64 + (g - 1) * 65 + 1:65])
                hp = Ab[:, 0:g * 65] \
                    .rearrange("p (g c) -> p g c", c=65)[:, :, 0:64]
                rcb = rc_b[:].unsqueeze(2).broadcast_to([128, g, 64])
                nc.vector.tensor_tensor(
                    y_b[:].rearrange("p (g c) -> p g c", c=64),
                    hp, rcb, ALU.mult)
                nc.scalar.activation(
                    W_b[:], y_b[:], AF.Exp, bias=0.0, scale=1.0)
                nc.vector.tensor_scalar(
                    t1_b[:], W_b[:], -1.0, 0.0, ALU.add, ALU.min)
                nc.vector.tensor_tensor(
                    h_b[:], t1_b[:], y_b[:], ALU.max)
                nc.sync.dma_start(
                    H_d.ap()[:, 448 * bi:448 * bi + g * 64], h_b[:])

    nc.compile()
    return nc


def _get_program():
    if "nc" not in _CACHED:
        _CACHED["nc"] = _build_program()
    return _CACHED["nc"]


def _host_prep(A, X, Ws, a):
    """Per-core host-side input preparation (cheap: ~67 MFLOP total)."""
    f64 = np.float64
    bf16 = ml_dtypes.bfloat16
    in_maps = []
    ci = (C_MASK * np.eye(128)).astype(bf16)
    for b in range(B):
        Wh = X[b].astype(f64) @ Ws.astype(f64)            # [N, D]
        Wh1 = (Wh @ a[:D].astype(f64))[:, 0]              # [N]
        Wh2 = (Wh @ a[D:].astype(f64))[:, 0]              # [N]
        S = max(0.0, float(Wh1.max() + Wh2.max()) - 10.5)
        wh1_hi = Wh1.astype(bf16)
        wh1_lo = (Wh1 - wh1_hi.astype(f64)).astype(bf16)
        wz = Wh2 - C_MASK
        wz_hi = wz.astype(bf16)
        wz_lo = (wz - wz_hi.astype(f64)).astype(bf16)

        blobS = np.zeros((128, 128), np.float32)
        blobS[:, 0] = -S
        blobS.view(np.uint16)[:, 128:256] = ci.view(np.uint16)

        blob2 = np.zeros((4, 4096), bf16)
        blob2[0, 0:2048] = wh1_hi
        blob2[1, 0:2048] = wh1_lo
        blob2[2:4, 0:2048] = np.ones((2, 2048), bf16)
        blob2[0:2, 2048:4096] = np.ones((2, 2048), bf16)
        blob2[2, 2048:4096] = wz_hi
        blob2[3, 2048:4096] = wz_lo

        whaug = np.ones((N, D + 1), np.float16)
        whaug[:, :D] = Wh.astype(np.float16)
        blobW = whaug.reshape(NT, 128, D + 1).transpose(1, 0, 2) \
            .reshape(128, NT * (D + 1)).copy()

        in_maps.append({
            "A": np.ascontiguousarray(A[b]),
            "blobS": blobS,
            "blob2": blob2,
            "blobW": blobW,
        })
    return in_maps


def kernel(A, X, Ws, a, _trace=False, _trace_kwargs=None):
    A = np.asarray(A, np.float32)
    X = np.asarray(X, np.float32)
    Ws = np.asarray(Ws, np.float32)
    a = np.asarray(a, np.float32)
    nc = _get_program()
    in_maps = _host_prep(A, X, Ws, a)
    kw = {}
    if _trace:
        kw = {"trace": True, **(_trace_kwargs or {})}
    res = bass_utils.run_bass_kernel_spmd(nc, in_maps, core_ids=list(range(B)), **kw)
    H = np.empty((B, N, D), np.float32)
    for b in range(B):
        Hd = np.asarray(res.results[b]["H"], np.float32)  # [128, 1024]
        for m in range(NT):
            c0 = 448 * (m // 7) + 64 * (m % 7)
            H[b, 128 * m:128 * (m + 1), :] = Hd[:, c0:c0 + 64]
    if _trace:
        kernel.last_results = res
    return H


# revision 11
# speedup vs baseline: 1.0801x; 1.0801x over previous
"""GAT layer (dense adjacency) on 8 Trainium2 NeuronCores.

Problem: H = elu(softmax_j(mask(A, leaky_relu(Wh1_i + Wh2_j))) @ Wh),
A: [8, 2048, 2048] 0/1 f32, X: [8, 2048, 64], Ws: [64, 64], a: [128, 1].

Sharding: data-parallel over batch B=8 -> one batch element per core.

Single pass over 16 j-slabs (A column tiles), paced by the A DMA
(~2.9 us per 1 MiB slab at 360 GB/s).  All per-slab compute is balanced
under the DMA pace by spreading it over four engines:
  - PE builds the full masked logits in PSUM (4 single-bank chunks of
    512 i-cols, ring of 5 banks):
        z[j, i] = Wh1[i] + (Wh2[j] - C) + C*A[i, j]
    via one K=4 matmul (rows: wh1_hi, wh1_lo broadcast by ones; ones
    rows scaled by (Wh2-C) hi/lo in the stationary operand) plus four
    K=128 matmuls with C*I against the bf16 high halves of the f32 A
    slab (the matmul transposes A; contraction depth is free).
  - leaky_relu(z) is computed chunk-parallel on ACT and DVE into one
    contiguous fp16 SBUF tile e_all[128, 2048]:
      cols [0:832]: ACT Prelu(z) (chunk 0 + first 320 of chunk 1)
      cols [832:2048]: DVE z2 = 0.2*z (fp16 out, the one 1x PSUM-reading
      pass), z5 = 5*z2 ~= z (TSP, 4x mode), e = max(z5, z2) (TT, 2x
      mode).  A direct (z*0.2) max z would need two PSUM reads in one
      op (PSUM has a single DVE read port), scalar_tensor_tensor runs
      at 1x only, and GPSIMD cannot run tensor ops at all.
  - The per-slab ACT queue order is Prelu(k) before Exp(k-1): the
    Prelus free this slab's PSUM ring tiles for the next slab's fills,
    keeping PE off the loop-carried critical path.
  - One ACT Exp over all 2048 cols: pa = exp(e_all - S) (fp16).  The
    masked entries (z ~= e - C) underflow to 0, implementing the mask.
  - H accumulates on PE in natural [i, d] layout: per i-tile m,
    acc[:, col(m):col(m)+65] += pa[:, 128m:128(m+1)]^T @ [Wh | 1]
    (ones column gives the softmax row sums).  3 PSUM banks, 7/7/2.
  - The last slab splits its Exp into three pieces aligned with the
    accumulator banks so each bank's epilogue starts early.
  - Epilogue per bank: strided reciprocals -> rc; y = acc * rc (DVE);
    elu via elu(y) = max(min(e^y - 1, 0), y): Exp (ACT), min (DVE TSP),
    max (DVE TT); one fp16 output DMA per bank into a packed [128, 1024]
    DRAM tensor (2 KiB rows, full DMA bandwidth), unpacked on the host.
"""
import sys

for _p in ("/opt/trn_rl_repo",):
    if _p not in sys.path:
        sys.path.append(_p)

import numpy as np
import ml_dtypes

import concourse.bass as bass
import concourse.bacc as bacc
import concourse.tile as tile
from concourse import mybir
from concourse import bass_utils

F32 = mybir.dt.float32
BF16 = mybir.dt.bfloat16
FP16 = mybir.dt.float16
AF = mybir.ActivationFunctionType
ALU = mybir.AluOpType

B, N, F, D = 8, 2048, 64, 64
NT = N // 128          # 16 j-slabs / i-tiles
C_MASK = 512.0
ALPHA = 0.2

_CACHED = {}


def _acc_col(m):
    """Column offset of i-tile m inside the [128, 1536] PSUM accumulator
    (7 tiles in bank 0, 7 in bank 1, 2 in bank 2)."""
    return 512 * (m // 7) + 65 * (m % 7)


def _build_program():
    nc = bacc.Bacc("TRN2", target_bir_lowering=False, debug=False)

    A_d = nc.dram_tensor("A", [N, N], F32, kind="ExternalInput")
    blobS_d = nc.dram_tensor("blobS", [128, 128], F32, kind="ExternalInput")
    blob2_d = nc.dram_tensor("blob2", [4, 4096], BF16, kind="ExternalInput")
    blobW_d = nc.dram_tensor("blobW", [128, NT * (D + 1)], FP16,
                             kind="ExternalInput")
    H_d = nc.dram_tensor("H", [128, NT * D], FP16, kind="ExternalOutput")

    with tile.TileContext(nc) as tc:
        with tc.tile_pool(name="const", bufs=1) as cp, \
             tc.tile_pool(name="aslab", bufs=8) as ap_pool, \
             tc.tile_pool(name="work", bufs=3) as wp, \
             tc.tile_pool(name="outp", bufs=1) as op_pool, \
             tc.tile_pool(name="psP", bufs=5, space="PSUM") as psP, \
             tc.tile_pool(name="psA", bufs=1, space="PSUM") as psA:

            # ---- constants (3 DMAs; small ones first) ----
            cbS = cp.tile([128, 128], F32, name="cbS")
            nc.sync.dma_start(cbS[:], blobS_d.ap())
            cb2 = cp.tile([4, 4096], BF16, name="cb2")
            nc.sync.dma_start(cb2[:], blob2_d.ap())
            cbW = cp.tile([128, NT * (D + 1)], FP16, name="cbW")
            negS = cbS[:, 0:1]
            ci = cbS[:].bitcast(BF16)[:, 128:256]
            fillB = cb2[:, 0:2048]        # rows: wh1_hi, wh1_lo, 1, 1
            fillL = cb2[:, 2048:4096]     # rows: 1, 1, wz_hi, wz_lo
            alpha02 = cp.tile([128, 1], F32, name="alpha02")
            nc.vector.memset(alpha02[:], ALPHA)
            # preload the ACT table set during input DMA
            warm = cp.tile([1, 1], F32, name="warm")
            nc.vector.memset(warm[:], 0.0)
            warm2 = cp.tile([1, 1], F32, name="warm2")
            nc.scalar.activation(warm2[:], warm[:], AF.Exp, bias=0.0, scale=1.0)

            # H^pre accumulator (+ row sums), [i, d] layout, 3 banks.
            # Zeroed once; the accumulating matmuls all use start=False
            # (start=True resets the whole PSUM bank, wiping the slab-0
            # contribution of co-resident regions).
            accs = [psA.tile([128, 512], F32, name=f"acc{b3}")
                    for b3 in range(3)]
            for b3 in range(3):
                nc.vector.memset(accs[b3][:], 0.0)

            aslabs = {}
            state = {}

            def dma_slab(t, split=False):
                sl = ap_pool.tile([128, N], F32, name=f"aslab{t}", tag="aslab")
                # sl[p, 128*r + q] = A[128*r + p, 128*t + q]
                if split:  # 4 row-block chunks so fills can start early
                    for rr in range(4):
                        nc.sync.dma_start(
                            sl[:, 512 * rr:512 * (rr + 1)]
                            .rearrange("p (r q) -> p r q", q=128),
                            A_d.ap()[512 * rr:512 * (rr + 1),
                                     128 * t:128 * (t + 1)]
                            .rearrange("(r p) q -> p r q", p=128),
                        )
                else:
                    nc.sync.dma_start(
                        sl[:].rearrange("p (r q) -> p r q", q=128),
                        A_d.ap()[:, 128 * t:128 * (t + 1)]
                        .rearrange("(r p) q -> p r q", p=128),
                    )
                aslabs[t] = sl

            def fills(t):
                # 4 single-bank chunks of 512 i-cols each; chunk c gets
                # z = Wh1[i] + (Wh2[j]-C) via one K=4 matmul, then + C*A
                # via 4 K=128 matmuls on the bf16 high halves of A.
                chunks = []
                lhs_t = fillL[:, 128 * t:128 * (t + 1)]
                for c in range(4):
                    pp = psP.tile([128, 512], F32, name=f"pp{c}_{t}", tag="pp")
                    nc.tensor.matmul(
                        pp[:], lhs_t, fillB[:, 512 * c:512 * (c + 1)],
                        start=True, stop=False,
                    )
                    for k4 in range(4):
                        r = 4 * c + k4
                        nc.tensor.matmul(
                            pp[:, 128 * k4:128 * (k4 + 1)],
                            aslabs[t][:].bitcast(BF16)[:, 256 * r + 1:256 * (r + 1):2],
                            ci[:],
                            start=False, stop=True, skip_group_check=True,
                        )
                    chunks.append(pp)
                return chunks

            def leaky(t, pp):
                # e_all[128, 2048] fp16 = leaky_relu(z), assembled
                # chunk-parallel across ACT and DVE.
                e_all = wp.tile([128, 2048], FP16, name=f"ea{t}", tag="ea")
                nc.scalar.activation(
                    e_all[:, 0:512], pp[0][:], AF.Prelu,
                    bias=0.0, scale=1.0, alpha=alpha02[:])
                nc.scalar.activation(
                    e_all[:, 512:832], pp[1][:, 0:320], AF.Prelu,
                    bias=0.0, scale=1.0, alpha=alpha02[:])
                for (c, c0, c1) in ((1, 320, 512), (2, 0, 512), (3, 0, 512)):
                    w = c1 - c0
                    z2 = wp.tile([128, w], FP16, name=f"z2_{t}_{c}",
                                 tag=f"z2{c}")
                    nc.vector.tensor_scalar(
                        z2[:], pp[c][:, c0:c1], ALPHA, None, ALU.mult)
                    z5 = wp.tile([128, w], FP16, name=f"z5_{t}_{c}",
                                 tag=f"z5{c}")
                    nc.vector.tensor_scalar(
                        z5[:], z2[:], 1.0 / ALPHA, None, ALU.mult)
                    nc.vector.tensor_tensor(
                        e_all[:, 512 * c + c0:512 * c + c1],
                        z5[:], z2[:], ALU.max)
                state[t] = e_all

            def exp(t):
                # pa = exp(e_all - S); last slab split on accumulator-bank
                # boundaries (i-tiles 0-6 / 7-13 / 14-15).
                e_all = state.pop(t)
                pa = wp.tile([128, 2048], FP16, name=f"pa{t}", tag="pa",
                             bufs=4)
                if t == NT - 1:
                    for (c0, c1) in ((0, 896), (896, 1792), (1792, 2048)):
                        nc.scalar.activation(
                            pa[:, c0:c1], e_all[:, c0:c1], AF.Exp,
                            bias=negS, scale=1.0)
                else:
                    nc.scalar.activation(
                        pa[:], e_all[:], AF.Exp, bias=negS, scale=1.0)
                return pa

            def accum(t, pa):
                for m in range(NT):
                    c0 = 65 * (m % 7)
                    nc.tensor.matmul(
                        accs[m // 7][:, c0:c0 + D + 1],
                        pa[:, 128 * m:128 * (m + 1)],
                        cbW[:, (D + 1) * t:(D + 1) * (t + 1)],
                        start=False, stop=(t == NT - 1),
                        skip_group_check=True,
                    )

            # ---- software-pipelined main loop ----
            # Lag structure: at step k the ACT queue runs Exp(k-1) then
            # Prelu(k); PE runs fills(k) then accum(k-3); DVE/Pool run
            # leaky(k).  DMA leads by 4 slabs.
            pas = {}
            dma_slab(0, split=True)
            for t in range(1, 4):
                dma_slab(t)
            nc.sync.dma_start(cbW[:], blobW_d.ap())
            for k in range(NT + 4):
                if k + 4 < NT:
                    dma_slab(k + 4)
                if k < NT:
                    pp = fills(k)
                    leaky(k, pp)
                if 1 <= k <= NT:
                    pas[k - 1] = exp(k - 1)
                if 3 <= k < NT + 3:
                    accum(k - 3, pas.pop(k - 3))

            # ---- epilogue per accumulator bank:
            # H = max(min(e^y - 1, 0), y), y = H_pre * (1/s) ----
            banks = [(0, 7), (7, 7), (14, 2)]  # (first tile, count)
            for bi, (m0, g) in enumerate(banks):
                Ab = accs[bi]
                rc_b = op_pool.tile([128, g], F32, name=f"rc{bi}")
                y_b = op_pool.tile([128, g * 64], FP16, name=f"y{bi}")
                W_b = op_pool.tile([128, g * 64], FP16, name=f"W{bi}")
                t1_b = op_pool.tile([128, g * 64], FP16, name=f"t1{bi}")
                h_b = op_pool.tile([128, g * 64], FP16, name=f"h{bi}")
                nc.vector.reciprocal(
                    rc_b[:], Ab[:, 64:64 + (g - 1) * 65 + 1:65])
                hp = Ab[:, 0:g * 65] \
                    .rearrange("p (g c) -> p g c", c=65)[:, :, 0:64]
                rcb = rc_b[:].unsqueeze(2).broadcast_to([128, g, 64])
                nc.vector.tensor_tensor(
                    y_b[:].rearrange("p (g c) -> p g c", c=64),
                    hp, rcb, ALU.mult)
                nc.scalar.activation(
                    W_b[:], y_b[:], AF.Exp, bias=0.0, scale=1.0)
                nc.vector.tensor_scalar(
                    t1_b[:], W_b[:], -1.0, 0.0, ALU.add, ALU.min)
                nc.vector.tensor_tensor(
                    h_b[:], t1_b[:], y_b[:], ALU.max)
                nc.sync.dma_start(
                    H_d.ap()[:, 448 * bi:448 * bi + g * 64], h_b[:])

    nc.compile()
    return nc


def _get_program():
    if "nc" not in _CACHED:
        _CACHED["nc"] = _build_program()
    return _CACHED["nc"]


def _host_prep(A, X, Ws, a):
    """Per-core host-side input preparation (cheap: ~67 MFLOP total)."""
    f64 = np.float64
    bf16 = ml_dtypes.bfloat16
    in_maps = []
    ci = (C_MASK * np.eye(128)).astype(bf16)
    for b in range(B):
        Wh = X[b].astype(f64) @ Ws.astype(f64)            # [N, D]
        Wh1 = (Wh @ a[:D].astype(f64))[:, 0]              # [N]
        Wh2 = (Wh @ a[D:].astype(f64))[:, 0]              # [N]
        S = max(0.0, float(Wh1.max() + Wh2.max()) - 10.5)
        wh1_hi = Wh1.astype(bf16)
        wh1_lo = (Wh1 - wh1_hi.astype(f64)).astype(bf16)
        wz = Wh2 - C_MASK
        wz_hi = wz.astype(bf16)
        wz_lo = (wz - wz_hi.astype(f64)).astype(bf16)

        blobS = np.zeros((128, 128), np.float32)
        blobS[:, 0] = -S
        blobS.view(np.uint16)[:, 128:256] = ci.view(np.uint16)

        blob2 = np.zeros((4, 4096), bf16)
        blob2[0, 0:2048] = wh1_hi
        blob2[1, 0:2048] = wh1_lo
        blob2[2:4, 0:2048] = np.ones((2, 2048), bf16)
        blob2[0:2, 2048:4096] = np.ones((2, 2048), bf16)
        blob2[2, 2048:4096] = wz_hi
        blob2[3, 2048:4096] = wz_lo

        whaug = np.ones((N, D + 1), np.float16)
        whaug[:, :D] = Wh.astype(np.float16)
        blobW = whaug.reshape(NT, 128, D + 1).transpose(1, 0, 2) \
            .reshape(128, NT * (D + 1)).copy()

        in_maps.append({
            "A": np.ascontiguousarray(A[b]),
            "blobS": blobS,
            "blob2": blob2,
            "blobW": blobW,
        })
    return in_maps


def kernel(A, X, Ws, a, _trace=False, _trace_kwargs=None):
    A = np.asarray(A, np.float32)
    X = np.asarray(X, np.float32)
    Ws = np.asarray(Ws, np.float32)
    a = np.asarray(a, np.float32)
    nc = _get_program()
    in_maps = _host_prep(A, X, Ws, a)
    kw = {}
    if _trace:
        kw = {"trace": True, **(_trace_kwargs or {})}
    res = bass_utils.run_bass_kernel_spmd(nc, in_maps, core_ids=list(range(B)), **kw)
    H = np.empty((B, N, D), np.float32)
    for b in range(B):
        Hd = np.asarray(res.results[b]["H"], np.float32)  # [128, 1024]
        for m in range(NT):
            c0 = 448 * (m // 7) + 64 * (m % 7)
            H[b, 128 * m:128 * (m + 1), :] = Hd[:, c0:c0 + 64]
    if _trace:
        kernel.last_results = res
    return H


# revision 16
# speedup vs baseline: 1.0979x; 1.0166x over previous
"""GAT layer (dense adjacency) on 8 Trainium2 NeuronCores.

Problem: H = elu(softmax_j(mask(A, leaky_relu(Wh1_i + Wh2_j))) @ Wh),
A: [8, 2048, 2048] 0/1 f32, X: [8, 2048, 64], Ws: [64, 64], a: [128, 1].

Sharding: data-parallel over batch B=8 -> one batch element per core.

Single pass over 16 j-slabs (A column tiles), paced by the A DMA
(~2.9 us per 1 MiB slab at 360 GB/s).  All per-slab compute is balanced
under the DMA pace by spreading it over four engines:
  - PE builds the full masked logits in PSUM (4 single-bank chunks of
    512 i-cols, ring of 5 banks):
        z[j, i] = Wh1[i] + (Wh2[j] - C) + C*A[i, j]
    via one K=4 matmul (rows: wh1_hi, wh1_lo broadcast by ones; ones
    rows scaled by (Wh2-C) hi/lo in the stationary operand) plus four
    K=128 matmuls with C*I against the bf16 high halves of the f32 A
    slab (the matmul transposes A; contraction depth is free).
  - leaky_relu(z) is computed chunk-parallel on ACT and DVE into one
    contiguous fp16 SBUF tile e_all[128, 2048]:
      cols [0:832]: ACT Prelu(z) (chunk 0 + first 320 of chunk 1)
      cols [832:2048]: DVE z2 = 0.2*z (fp16 out, the one 1x PSUM-reading
      pass), z5 = 5*z2 ~= z (TSP, 4x mode), e = max(z5, z2) (TT, 2x
      mode).  A direct (z*0.2) max z would need two PSUM reads in one
      op (PSUM has a single DVE read port), scalar_tensor_tensor runs
      at 1x only, and GPSIMD cannot run tensor ops at all.
  - The per-slab ACT queue order is Prelu(k) before Exp(k-1): the
    Prelus free this slab's PSUM ring tiles for the next slab's fills,
    keeping PE off the loop-carried critical path.
  - One ACT Exp over all 2048 cols: pa = exp(e_all - S) (fp16).  The
    masked entries (z ~= e - C) underflow to 0, implementing the mask.
  - H accumulates on PE in natural [i, d] layout: per i-tile m,
    acc[:, col(m):col(m)+65] += pa[:, 128m:128(m+1)]^T @ [Wh | 1]
    (ones column gives the softmax row sums).  3 PSUM banks, 7/7/2.
  - The last slab splits its Exp into three pieces aligned with the
    accumulator banks so each bank's epilogue starts early.
  - Epilogue per bank: strided reciprocals -> rc; y = acc * rc (DVE);
    elu via elu(y) = max(min(e^y - 1, 0), y): Exp (ACT), min (DVE TSP),
    max (DVE TT); one fp16 output DMA per bank into a packed [128, 1024]
    DRAM tensor (2 KiB rows, full DMA bandwidth), unpacked on the host.
"""
import sys

for _p in ("/opt/trn_rl_repo",):
    if _p not in sys.path:
        sys.path.append(_p)

import numpy as np
import ml_dtypes

import concourse.bass as bass
import concourse.bacc as bacc
import concourse.tile as tile
from concourse import mybir
from concourse import bass_utils

F32 = mybir.dt.float32
BF16 = mybir.dt.bfloat16
FP16 = mybir.dt.float16
AF = mybir.ActivationFunctionType
ALU = mybir.AluOpType

B, N, F, D = 8, 2048, 64, 64
NT = N // 128          # 16 j-slabs / i-tiles
C_MASK = 512.0
ALPHA = 0.2

_CACHED = {}


def _acc_col(m):
    """Column offset of i-tile m inside the [128, 1536] PSUM accumulator
    (7 tiles in bank 0, 7 in bank 1, 2 in bank 2)."""
    return 512 * (m // 7) + 65 * (m % 7)


def _build_program():
    nc = bacc.Bacc("TRN2", target_bir_lowering=False, debug=False)

    A_d = nc.dram_tensor("A", [N, N], F32, kind="ExternalInput")
    blobS_d = nc.dram_tensor("blobS", [128, 128], F32, kind="ExternalInput")
    blob2_d = nc.dram_tensor("blob2", [4, 4096], BF16, kind="ExternalInput")
    blobW_d = nc.dram_tensor("blobW", [128, NT * (D + 1)], FP16,
                             kind="ExternalInput")
    H_d = nc.dram_tensor("H", [128, NT * D], FP16, kind="ExternalOutput")

    with tile.TileContext(nc) as tc:
        with tc.tile_pool(name="const", bufs=1) as cp, \
             tc.tile_pool(name="aslab", bufs=8) as ap_pool, \
             tc.tile_pool(name="work", bufs=3) as wp, \
             tc.tile_pool(name="outp", bufs=1) as op_pool, \
             tc.tile_pool(name="psP", bufs=5, space="PSUM") as psP, \
             tc.tile_pool(name="psA", bufs=1, space="PSUM") as psA:

            # ---- constants (issued on ACT/DVE HWDGE queues so they do
            # not delay the slab-0 A DMA on the SP queue) ----
            cbS = cp.tile([128, 128], F32, name="cbS")
            nc.scalar.dma_start(cbS[:], blobS_d.ap())
            cb2 = cp.tile([4, 4096], BF16, name="cb2")
            nc.scalar.dma_start(cb2[:], blob2_d.ap())
            cbW = cp.tile([128, NT * (D + 1)], FP16, name="cbW")
            negS = cbS[:, 0:1]
            ci = cbS[:].bitcast(BF16)[:, 128:256]
            fillB = cb2[:, 0:2048]        # rows: wh1_hi, wh1_lo, 1, 1
            fillL = cb2[:, 2048:4096]     # rows: 1, 1, wz_hi, wz_lo
            alpha02 = cp.tile([128, 1], F32, name="alpha02")
            nc.vector.memset(alpha02[:], ALPHA)
            # preload the ACT table set during input DMA
            warm = cp.tile([1, 1], F32, name="warm")
            nc.vector.memset(warm[:], 0.0)
            warm2 = cp.tile([1, 1], F32, name="warm2")
            nc.scalar.activation(warm2[:], warm[:], AF.Exp, bias=0.0, scale=1.0)
            # warm the PE p-state ramp during the input DMA (~3 us of
            # back-to-back matmuls brings the cost model to full clock)
            wmm = cp.tile([1, 512], BF16, name="wmm")
            nc.vector.memset(wmm[:], 0.0)

            # H^pre accumulator (+ row sums), [i, d] layout, 3 banks.
            # Zeroed once; the accumulating matmuls all use start=False
            # (start=True resets the whole PSUM bank, wiping the slab-0
            # contribution of co-resident regions).
            accs = [psA.tile([128, 512], F32, name=f"acc{b3}")
                    for b3 in range(3)]
            for b3 in range(3):
                nc.vector.memset(accs[b3][:], 0.0)

            # PE warm-up: ~12 dependency-free matmuls into a scratch
            # PSUM tile while the first A slab is still in flight.
            warmpp = psP.tile([128, 512], F32, name="warmpp", tag="pp")
            for _w in range(12):
                nc.tensor.matmul(
                    warmpp[:], wmm[:, 0:128], wmm[:],
                    start=True, stop=True)

            aslabs = {}
            state = {}

            def dma_slab(t, split=False):
                sl = ap_pool.tile([128, N], F32, name=f"aslab{t}", tag="aslab")
                # sl[p, 128*r + q] = A[128*r + p, 128*t + q]
                if split:  # 4 row-block chunks so fills can start early
                    for rr in range(4):
                        nc.sync.dma_start(
                            sl[:, 512 * rr:512 * (rr + 1)]
                            .rearrange("p (r q) -> p r q", q=128),
                            A_d.ap()[512 * rr:512 * (rr + 1),
                                     128 * t:128 * (t + 1)]
                            .rearrange("(r p) q -> p r q", p=128),
                        )
                else:
                    nc.sync.dma_start(
                        sl[:].rearrange("p (r q) -> p r q", q=128),
                        A_d.ap()[:, 128 * t:128 * (t + 1)]
                        .rearrange("(r p) q -> p r q", p=128),
                    )
                aslabs[t] = sl

            def fills(t):
                # 4 single-bank chunks of 512 i-cols each; chunk c gets
                # z = Wh1[i] + (Wh2[j]-C) via one K=4 matmul, then + C*A
                # via 4 K=128 matmuls on the bf16 high halves of A.
                chunks = []
                lhs_t = fillL[:, 128 * t:128 * (t + 1)]
                for c in range(4):
                    pp = psP.tile([128, 512], F32, name=f"pp{c}_{t}", tag="pp")
                    nc.tensor.matmul(
                        pp[:], lhs_t, fillB[:, 512 * c:512 * (c + 1)],
                        start=True, stop=False,
                    )
                    for k4 in range(4):
                        r = 4 * c + k4
                        nc.tensor.matmul(
                            pp[:, 128 * k4:128 * (k4 + 1)],
                            aslabs[t][:].bitcast(BF16)[:, 256 * r + 1:256 * (r + 1):2],
                            ci[:],
                            start=False, stop=True, skip_group_check=True,
                        )
                    chunks.append(pp)
                return chunks

            def leaky(t, pp):
                # e_all[128, 2048] fp16 = leaky_relu(z), assembled
                # chunk-parallel across ACT and DVE.
                e_all = wp.tile([128, 2048], FP16, name=f"ea{t}", tag="ea")
                nc.scalar.activation(
                    e_all[:, 0:512], pp[0][:], AF.Prelu,
                    bias=0.0, scale=1.0, alpha=alpha02[:])
                nc.scalar.activation(
                    e_all[:, 512:832], pp[1][:, 0:320], AF.Prelu,
                    bias=0.0, scale=1.0, alpha=alpha02[:])
                for (c, c0, c1) in ((1, 320, 512), (2, 0, 512), (3, 0, 512)):
                    w = c1 - c0
                    z2 = wp.tile([128, w], FP16, name=f"z2_{t}_{c}",
                                 tag=f"z2{c}")
                    nc.vector.tensor_scalar(
                        z2[:], pp[c][:, c0:c1], ALPHA, None, ALU.mult)
                    z5 = wp.tile([128, w], FP16, name=f"z5_{t}_{c}",
                                 tag=f"z5{c}")
                    nc.vector.tensor_scalar(
                        z5[:], z2[:], 1.0 / ALPHA, None, ALU.mult)
                    nc.vector.tensor_tensor(
                        e_all[:, 512 * c + c0:512 * c + c1],
                        z5[:], z2[:], ALU.max)
                state[t] = e_all

            def exp(t):
                # pa = exp(e_all - S); last slab split on accumulator-bank
                # boundaries (i-tiles 0-6 / 7-13 / 14-15).
                e_all = state.pop(t)
                pa = wp.tile([128, 2048], FP16, name=f"pa{t}", tag="pa",
                             bufs=4)
                if t == NT - 1:
                    for (c0, c1) in ((0, 896), (896, 1792), (1792, 2048)):
                        nc.scalar.activation(
                            pa[:, c0:c1], e_all[:, c0:c1], AF.Exp,
                            bias=negS, scale=1.0)
                else:
                    nc.scalar.activation(
                        pa[:], e_all[:], AF.Exp, bias=negS, scale=1.0)
                return pa

            def accum(t, pa):
                for m in range(NT):
                    c0 = 65 * (m % 7)
                    nc.tensor.matmul(
                        accs[m // 7][:, c0:c0 + D + 1],
                        pa[:, 128 * m:128 * (m + 1)],
                        cbW[:, (D + 1) * t:(D + 1) * (t + 1)],
                        start=False, stop=(t == NT - 1),
                        skip_group_check=True,
                    )

            # ---- software-pipelined main loop ----
            # Lag structure: at step k the ACT queue runs Exp(k-1) then
            # Prelu(k); PE runs fills(k) then accum(k-3); DVE/Pool run
            # leaky(k).  DMA leads by 4 slabs.
            pas = {}
            dma_slab(0, split=True)
            for t in range(1, 4):
                dma_slab(t)
            nc.sync.dma_start(cbW[:], blobW_d.ap())
            for k in range(NT + 4):
                if k + 4 < NT:
                    # the last slab is split by row-blocks so its chunks
                    # can be processed while the rest is still in flight
                    dma_slab(k + 4, split=(k + 4 == NT - 1))
                if k < NT:
                    pp = fills(k)
                    leaky(k, pp)
                if 1 <= k <= NT:
                    pas[k - 1] = exp(k - 1)
                if 3 <= k < NT + 3:
                    accum(k - 3, pas.pop(k - 3))

            # ---- epilogue per accumulator bank:
            # H = max(min(e^y - 1, 0), y), y = H_pre * (1/s) ----
            banks = [(0, 7), (7, 7), (14, 2)]  # (first tile, count)
            for bi, (m0, g) in enumerate(banks):
                Ab = accs[bi]
                rc_b = op_pool.tile([128, g], F32, name=f"rc{bi}")
                y_b = op_pool.tile([128, g * 64], FP16, name=f"y{bi}")
                W_b = op_pool.tile([128, g * 64], FP16, name=f"W{bi}")
                t1_b = op_pool.tile([128, g * 64], FP16, name=f"t1{bi}")
                h_b = op_pool.tile([128, g * 64], FP16, name=f"h{bi}")
                nc.vector.reciprocal(
                    rc_b[:], Ab[:, 64:64 + (g - 1) * 65 + 1:65])
                hp = Ab[:, 0:g * 65] \
                    .rearrange("p (g c) -> p g c", c=65)[:, :, 0:64]
                rcb = rc_b[:].unsqueeze(2).broadcast_to([128, g, 64])
                nc.vector.tensor_tensor(
                    y_b[:].rearrange("p (g c) -> p g c", c=64),
                    hp, rcb, ALU.mult)
                nc.scalar.activation(
                    W_b[:], y_b[:], AF.Exp, bias=0.0, scale=1.0)
                nc.vector.tensor_scalar(
                    t1_b[:], W_b[:], -1.0, 0.0, ALU.add, ALU.min)
                nc.vector.tensor_tensor(
                    h_b[:], t1_b[:], y_b[:], ALU.max)
                nc.sync.dma_start(
                    H_d.ap()[:, 448 * bi:448 * bi + g * 64], h_b[:])

    nc.compile()
    return nc


def _get_program():
    if "nc" not in _CACHED:
        _CACHED["nc"] = _build_program()
    return _CACHED["nc"]


def _host_prep(A, X, Ws, a):
    """Per-core host-side input preparation (cheap: ~67 MFLOP total)."""
    f64 = np.float64
    bf16 = ml_dtypes.bfloat16
    in_maps = []
    ci = (C_MASK * np.eye(128)).astype(bf16)
    for b in range(B):
        Wh = X[b].astype(f64) @ Ws.astype(f64)            # [N, D]
        Wh1 = (Wh @ a[:D].astype(f64))[:, 0]              # [N]
        Wh2 = (Wh @ a[D:].astype(f64))[:, 0]              # [N]
        S = max(0.0, float(Wh1.max() + Wh2.max()) - 10.5)
        wh1_hi = Wh1.astype(bf16)
        wh1_lo = (Wh1 - wh1_hi.astype(f64)).astype(bf16)
        wz = Wh2 - C_MASK
        wz_hi = wz.astype(bf16)
        wz_lo = (wz - wz_hi.astype(f64)).astype(bf16)

        blobS = np.zeros((128, 128), np.float32)
        blobS[:, 0] = -S
        blobS.view(np.uint16)[:, 128:256] = ci.view(np.uint16)

        blob2 = np.zeros((4, 4096), bf16)
        blob2[0, 0:2048] = wh1_hi
        blob2[1, 0:2048] = wh1_lo
        blob2[2:4, 0:2048] = np.ones((2, 2048), bf16)
        blob2[0:2, 2048:4096] = np.ones((2, 2048), bf16)
        blob2[2, 2048:4096] = wz_hi
        blob2[3, 2048:4096] = wz_lo

        whaug = np.ones((N, D + 1), np.float16)
        whaug[:, :D] = Wh.astype(np.float16)
        blobW = whaug.reshape(NT, 128, D + 1).transpose(1, 0, 2) \
            .reshape(128, NT * (D + 1)).copy()

        in_maps.append({
            "A": np.ascontiguousarray(A[b]),
            "blobS": blobS,
            "blob2": blob2,
            "blobW": blobW,
        })
    return in_maps


def kernel(A, X, Ws, a, _trace=False, _trace_kwargs=None):
    A = np.asarray(A, np.float32)
    X = np.asarray(X, np.float32)
    Ws = np.asarray(Ws, np.float32)
    a = np.asarray(a, np.float32)
    nc = _get_program()
    in_maps = _host_prep(A, X, Ws, a)
    kw = {}
    if _trace:
        kw = {"trace": True, **(_trace_kwargs or {})}
    res = bass_utils.run_bass_kernel_spmd(nc, in_maps, core_ids=list(range(B)), **kw)
    H = np.empty((B, N, D), np.float32)
    for b in range(B):
        Hd = np.asarray(res.results[b]["H"], np.float32)  # [128, 1024]
        for m in range(NT):
            c0 = 448 * (m // 7) + 64 * (m % 7)
            H[b, 128 * m:128 * (m + 1), :] = Hd[:, c0:c0 + 64]
    if _trace:
        kernel.last_results = res
    return H
